# revision 1
# baseline (speedup 1.0000x reference)
"""Trainium2 Bass kernel for nn_CA3RecurrentMatrix (scatter_memory).

Math: the reference's Ben-Israel-Cohen pseudoinverse iteration collapses
algebraically.  With pinv_0 = alpha*A^T, every iterate has the form
pinv_n = P_n(G) A^T with G = A^T A (C x C) and P_{n+1} = 2P_n - P_n G P_n.
The final output is query @ (P_8 G).  On the eigenvalues g of G the map is
u_8 = 1 - (1 - alpha*g)^256 = 256(alpha g) - C(256,2)(alpha g)^2 + ...
Because alpha <= 5e-4/||A||_F^2 and g_max/||A||_F^2 ~ (sqrt(K)+sqrt(C))^2/(K*C),
alpha*g_max <= ~7.2e-7, so the cubic term is < 1e-8 relative -- below fp32
noise.  Hence exactly (to fp32):

    M   = 256*alpha*G - 32640*alpha^2*G^2
    out = query @ M

(The reference's masked early-stop never fires: its residual stays ~||A||_F,
far above tol=1e-4, for any input of this shape/scale.)

Distribution over 8 cores: core i computes the row block G[R_i,:] = W_i^T A
(W_i = A[:, R_i]) in float32r, AllGathers G in bf16 (only consumed by the
G^2 term, whose weight in M is ~9e-5), transposes its block on-chip,
computes G^2[R_i,:], combines with c1*G[R_i,:] (full fp32) into M[R_i,:],
AllGathers M (f32r payload) in two pipelined chunks, then computes its 1/8
slice of the query batch: out_i = Q_i @ M.  ||A||_F^2 is computed from the
local W shard and shared via a tiny AllGather so the alpha chain runs
concurrently with GEMM1 instead of after the big AllGather.
"""
import sys, os, types

sys.path.insert(0, "/opt/trn_rl_repo")

import numpy as np

B, C, K = 8192, 2048, 4096
NCORES = 8
CB = C // NCORES     # 256 G-row block per core
BB = B // NCORES     # 1024 query rows per core
ALPHA_CLAMP = 5e-4
C1 = 256.0           # C(256,1)
C2 = -32640.0        # -C(256,2)

_CACHE = {}


def _install_ntff_shim():
    """Make trace=True work under axon (antenv.axon_hooks is absent here)."""
    if "antenv.axon_hooks" in sys.modules:
        return
    try:
        import antenv
    except ImportError:
        return
    mod = types.ModuleType("antenv.axon_hooks")
    state = {"hook": None, "resolved": False}

    def set_axon_ntff_profile_hook(hook):
        state["hook"], state["resolved"] = hook, True

    def get_axon_ntff_profile_hook():
        if not state["resolved"]:
            state["resolved"] = True
            try:
                if "/root/.axon_site" not in sys.path:
                    sys.path.insert(0, "/root/.axon_site")
                from trn_agent_boot.trn_boot import _ntff_profile_via_ctypes
                state["hook"] = _ntff_profile_via_ctypes("/opt/axon/libaxon_pjrt.so")
            except Exception:
                state["hook"] = None
        return state["hook"]

    mod.set_axon_ntff_profile_hook = set_axon_ntff_profile_hook
    mod.get_axon_ntff_profile_hook = get_axon_ntff_profile_hook
    sys.modules["antenv.axon_hooks"] = mod
    antenv.axon_hooks = mod


def build_nc():
    import concourse.bacc as bacc
    import concourse.mybir as mybir
    from concourse import tile

    f32 = mybir.dt.float32
    f32r = mybir.dt.float32r
    bf16 = mybir.dt.bfloat16
    RG = [list(range(NCORES))]

    nc = bacc.Bacc("TRN2", target_bir_lowering=False, debug=False,
                   num_devices=NCORES)
    a_d = nc.dram_tensor("a", (K, C), f32, kind="ExternalInput")
    w_d = nc.dram_tensor("w", (K, CB), f32, kind="ExternalInput")
    qt_d = nc.dram_tensor("qt", (C, BB), f32, kind="ExternalInput")
    ls_d = nc.dram_tensor("ls", (1, 1), f32, kind="ExternalInput")
    id_d = nc.dram_tensor("ident", (128, 128), f32, kind="ExternalInput")
    out_d = nc.dram_tensor("out", (BB, C), f32, kind="ExternalOutput")

    KT = K // 128    # 32 k-tiles over K
    CT = C // 128    # 16 tiles over C
    NB = C // 512    # 4 512-wide column blocks
    MB3 = BB // 128  # 8 output row tiles per core

    with tile.TileContext(nc) as tc:
        with tc.tile_pool(name="sbuf", bufs=1) as pool, \
             tc.tile_pool(name="psum", bufs=1, space="PSUM") as psum, \
             tc.tile_pool(name="dram", bufs=1, space="DRAM") as dram:
            gin = dram.tile([CB, C], f32)
            gout = dram.tile([C, C], f32, addr_space="Shared")
            min_t = dram.tile([CB, C], f32)
            mout = dram.tile([C, C], f32, addr_space="Shared")

            ident_sb = pool.tile([128, 128], f32, tag="ident")
            nc.gpsimd.dma_start(ident_sb[:], id_d.ap()[:, :])
            ls_sb = pool.tile([1, 1], f32, tag="ls")
            nc.gpsimd.dma_start(ls_sb[:], ls_d.ap()[:, :])

            # ---- GEMM1: G_rows = W^T A   [CB, C]; also wsq = per-tile sum w^2 ----
            with nc.named_scope("gemm1"):
                psg = []
                for j in range(8):
                    pt = psum.tile([128, 512], f32, tag=f"ps{j}", name=f"psg{j}")
                    psg.append(pt)
                for k in range(KT):
                    ak = pool.tile([128, C], f32r, tag="ak", bufs=3)
                    for q in range(4):
                        qeng = nc.sync if q % 2 == 0 else nc.scalar
                        qeng.dma_start(
                            ak[:, q * 512:(q + 1) * 512],
                            a_d.ap()[k * 128:(k + 1) * 128,
                                     q * 512:(q + 1) * 512].bitcast(f32r))
                    wk = pool.tile([128, CB], f32r, tag="wk", bufs=4)
                    dma_eng = nc.sync if k % 2 == 0 else nc.scalar
                    dma_eng.dma_start(
                        wk[:], w_d.ap()[k * 128:(k + 1) * 128, :].bitcast(f32r))
                    for n in range(NB):
                        for m in range(2):
                            nc.tensor.matmul(
                                psg[m * NB + n][:],
                                wk[:, m * 128:(m + 1) * 128],
                                ak[:, n * 512:(n + 1) * 512],
                                start=(k == 0), stop=(k == KT - 1))
                g_rows = []
                for m in range(2):
                    gr = pool.tile([128, C], f32, tag=f"grows{m}")
                    for n in range(NB):
                        nc.vector.tensor_copy(
                            gr[:, n * 512:(n + 1) * 512], psg[m * NB + n][:])
                    nc.sync.dma_start(gin[m * 128:(m + 1) * 128, :], gr[:])
                    g_rows.append(gr)

            nc.gpsimd.collective_compute(
                "AllGather", mybir.AluOpType.bypass, replica_groups=RG,
                ins=[gin.opt()], outs=[gout.opt()])

            # ---- alpha chain: fro2 = tr(G) from the gathered diagonal ----
            with nc.named_scope("alpha"):
                diag = pool.tile([16, 128], f32, tag="diag")
                flat = gout[:, :].rearrange("a b -> (a b)")
                for sdg in range(16):
                    off = 128 * sdg * (C + 1)
                    seg = flat[off:off + (C + 1) * 127 + 1:C + 1]
                    nc.scalar.dma_start(diag[sdg:sdg + 1, :], seg.unsqueeze(0))
                dpart = pool.tile([16, 1], f32, tag="dpart")
                nc.vector.reduce_sum(dpart[:], diag[:], axis=mybir.AxisListType.X)
                fro2 = pool.tile([1, 1], f32, tag="fro2")
                nc.gpsimd.tensor_reduce(fro2[:], dpart[:], op=mybir.AluOpType.add,
                                        axis=mybir.AxisListType.C)
                ex = pool.tile([1, 1], f32, tag="ex")
                nc.scalar.activation(ex[:], ls_sb[:],
                                     mybir.ActivationFunctionType.Exp)
                emin = pool.tile([1, 1], f32, tag="emin")
                nc.vector.tensor_scalar_min(emin[:], ex[:], ALPHA_CLAMP)
                den = pool.tile([1, 1], f32, tag="den")
                nc.vector.tensor_scalar_add(den[:], fro2[:], 1e-8)
                r0 = pool.tile([1, 1], f32, tag="r0")
                nc.vector.reciprocal(r0[:], den[:])
                # one Newton step: r = r0*(2 - den*r0)
                t1 = pool.tile([1, 1], f32, tag="t1")
                nc.vector.tensor_mul(t1[:], den[:], r0[:])
                t2 = pool.tile([1, 1], f32, tag="t2")
                nc.vector.tensor_scalar(t2[:], t1[:], -1.0, 2.0,
                                        op0=mybir.AluOpType.mult,
                                        op1=mybir.AluOpType.add)
                rr = pool.tile([1, 1], f32, tag="rr")
                nc.vector.tensor_mul(rr[:], r0[:], t2[:])
                al = pool.tile([1, 1], f32, tag="al")
                nc.vector.tensor_mul(al[:], emin[:], rr[:])
                al2 = pool.tile([1, 1], f32, tag="al2")
                nc.vector.tensor_mul(al2[:], al[:], al[:])
                c1s = pool.tile([1, 1], f32, tag="c1s")
                nc.vector.tensor_scalar_mul(c1s[:], al[:], C1)
                c2s = pool.tile([1, 1], f32, tag="c2s")
                nc.vector.tensor_scalar_mul(c2s[:], al2[:], C2)
                c1b = pool.tile([128, 1], f32, tag="c1b")
                nc.gpsimd.partition_broadcast(c1b[:], c1s[:])
                c2b = pool.tile([128, 1], f32, tag="c2b")
                nc.gpsimd.partition_broadcast(c2b[:], c2s[:])

            # query^T resident (f32r view); on the scalar queue so GEMM2's
            # rhs stream (sync queue) is never stuck behind it
            qt_sb = []
            for t in range(CT):
                qts = pool.tile([128, BB], f32r, tag=f"qt{t}", name=f"qts{t}")
                nc.scalar.dma_start(
                    qts[:], qt_d.ap()[t * 128:(t + 1) * 128, :].bitcast(f32r))
                qt_sb.append(qts)

            # c1*G_rows on DVE, overlapped with GEMM1 tail / AllGather wait
            tmpm = []
            for m in range(2):
                tm = pool.tile([128, C], f32, tag=f"tmpm{m}", name=f"tmpm{m}")
                nc.vector.tensor_scalar_mul(tm[:], g_rows[m][:], c1b[:])
                tmpm.append(tm)

            # ---- transpose own block: GT[t] = G[t*128:(t+1)*128, R_i] ----
            with nc.named_scope("transpose"):
                gt = []
                for t in range(CT):
                    gtt = pool.tile([128, CB], f32r, tag=f"gt{t}", name=f"gtt{t}")
                    for m in range(2):
                        tp = psum.tile([128, 128], f32,
                                       tag=f"ps{(t * 2 + m) % 8}", name=f"tp{t}_{m}")
                        nc.tensor.transpose(
                            tp[:], g_rows[m][:, t * 128:(t + 1) * 128], ident_sb[:])
                        nc.vector.tensor_copy(gtt[:, m * 128:(m + 1) * 128], tp[:])
                    gt.append(gtt)

            # ---- GEMM2: Z = (G[:,R_i])^T G = G^2[R_i,:]; M = c2*Z + c1*G ----
            with nc.named_scope("gemm2"):
                psg2 = []
                for j in range(8):
                    pt2 = psum.tile([128, 512], f32, tag=f"ps{j}", name=f"psg2{j}")
                    psg2.append(pt2)
                for t in range(CT):
                    grhs = pool.tile([128, C], f32r, tag="grhs", bufs=2)
                    nc.sync.dma_start(
                        grhs[:], gout[t * 128:(t + 1) * 128, :].bitcast(f32r))
                    for n in range(NB):
                        for m in range(2):
                            nc.tensor.matmul(
                                psg2[m * NB + n][:],
                                gt[t][:, m * 128:(m + 1) * 128],
                                grhs[:, n * 512:(n + 1) * 512],
                                start=(t == 0), stop=(t == CT - 1))
                for m in range(2):
                    msb = pool.tile([128, C], f32r, tag=f"grows{m}", name=f"msb{m}")
                    for n in range(NB):
                        sl = slice(n * 512, (n + 1) * 512)
                        zc = pool.tile([128, 512], f32, tag="zc", bufs=2)
                        nc.vector.tensor_copy(zc[:], psg2[m * NB + n][:])
                        nc.vector.tensor_scalar_mul(zc[:], zc[:], c2b[:])
                        nc.vector.tensor_add(msb[:, sl], zc[:], tmpm[m][:, sl])
                    nc.sync.dma_start(min_t[m * 128:(m + 1) * 128, :],
                                      msb[:].bitcast(f32))

            nc.gpsimd.collective_compute(
                "AllGather", mybir.AluOpType.bypass, replica_groups=RG,
                ins=[min_t.opt()], outs=[mout.opt()])

            # ---- GEMM3: out_i = Q_i @ M ----
            with nc.named_scope("gemm3"):
                for n in range(NB):
                    mr = []
                    for t in range(CT):
                        mrt = pool.tile([128, 512], f32r, tag=f"mr{t}", bufs=1,
                                        name=f"mrt{t}")
                        dma_eng = nc.sync if t % 2 == 0 else nc.scalar
                        dma_eng.dma_start(
                            mrt[:],
                            mout[t * 128:(t + 1) * 128,
                                 n * 512:(n + 1) * 512].bitcast(f32r))
                        mr.append(mrt)
                    for mp in range(MB3 // 2):
                        pos = []
                        for h in range(2):
                            po = psum.tile([128, 512], f32,
                                           tag=f"ps{(2 * mp + h) % 8}",
                                           name=f"po{n}_{mp}_{h}")
                            pos.append(po)
                        for t in range(CT):
                            for h in range(2):
                                m = 2 * mp + h
                                nc.tensor.matmul(
                                    pos[h][:],
                                    qt_sb[t][:, m * 128:(m + 1) * 128],
                                    mr[t][:], start=(t == 0),
                                    stop=(t == CT - 1))
                        for h in range(2):
                            m = 2 * mp + h
                            osb = pool.tile([128, 512], f32, tag="osb", bufs=2)
                            nc.vector.tensor_copy(osb[:], pos[h][:])
                            nc.scalar.dma_start(
                                out_d.ap()[m * 128:(m + 1) * 128,
                                           n * 512:(n + 1) * 512], osb[:])
    nc.compile()
    return nc


def _get_nc():
    if "nc" not in _CACHE:
        _CACHE["nc"] = build_nc()
    return _CACHE["nc"]


def _run(query, memory_mean, ben_israel_log_scale, trace=False, trace_cores=None):
    from concourse import bass_utils

    _install_ntff_shim()
    nc = _get_nc()

    q = np.asarray(query, dtype=np.float32)
    a = np.ascontiguousarray(np.asarray(memory_mean, dtype=np.float32))
    ls = np.asarray(ben_israel_log_scale, dtype=np.float32).reshape(1, 1)
    ident = np.eye(128, dtype=np.float32)

    in_maps = []
    for i in range(NCORES):
        in_maps.append({
            "a": a,
            "w": np.ascontiguousarray(a[:, i * CB:(i + 1) * CB]),
            "qt": np.ascontiguousarray(q[i * BB:(i + 1) * BB, :].T),
            "ls": ls,
            "ident": ident,
        })
    res = bass_utils.run_bass_kernel_spmd(
        nc, in_maps, core_ids=list(range(NCORES)), trace=trace,
        trace_cores=trace_cores)
    out = np.concatenate([res.results[i]["out"] for i in range(NCORES)], axis=0)
    return out, res


def kernel(query, memory_mean, ben_israel_log_scale):
    out, _ = _run(query, memory_mean, ben_israel_log_scale, trace=False)
    return out



# revision 3
# speedup vs baseline: 1.9821x; 1.9821x over previous
"""Trainium2 Bass kernel for nn_CA3RecurrentMatrix (scatter_memory).

Math: the reference's Ben-Israel-Cohen pseudoinverse iteration collapses
algebraically.  With pinv_0 = alpha*A^T, every iterate is pinv_n = p_n(G) A^T
with G = A^T A (C x C), and on eigenvalues g of G the output polynomial is
u_8(g)*g = 1 - (1 - alpha*g)^256 = 256*alpha*g - C(256,2)*(alpha*g)^2 + ...
Because alpha <= 5e-4/||A||_F^2, alpha*g_max ~ 7e-7, so even the QUADRATIC
term is <1e-4 relative (verified numerically: dropping it gives 5.7e-5 max
rel err).  Hence, to well within the 2e-2 gate:

    out = (256*alpha) * query @ (A^T A)

Distribution over 8 cores, all bf16 compute (validated 4.2e-3 max rel err):
core i holds W_i = A[:, R_i] (bf16) and computes G row-block
G[R_i, :] = W_i^T A in four 512-column chunks; each chunk is AllGathered
(bf16, 2MB) as soon as it is ready, and the big GEMM out_i = Q_i @ G
consumes gathered column-chunks as they land -- the tensor engine never
waits for a monolithic collective.  ||A||_F^2 comes from sum(W_i^2) on the
vector engine plus a 4-byte AllReduce, fully hidden under GEMM1; the scale
s = 256*min(exp(ls),5e-4)/(fro2+1e-8) is applied during the PSUM->SBUF
eviction of the output tiles.
"""
import sys, os, types

sys.path.insert(0, "/opt/trn_rl_repo")

import numpy as np

B, C, K = 8192, 2048, 4096
NCORES = 8
CB = C // NCORES     # 256 G-row block per core
BB = B // NCORES     # 1024 query rows per core
KT = K // 128        # 32 k-tiles over K
CT = C // 128        # 16 c-tiles over C
CW = 512             # column-chunk width
NCH = C // CW        # 4 column chunks
ALPHA_CLAMP = 5e-4
C1 = 256.0

_CACHE = {}


def _install_ntff_shim():
    """Make trace=True work under axon (antenv.axon_hooks is absent here)."""
    if "antenv.axon_hooks" in sys.modules:
        return
    try:
        import antenv
    except ImportError:
        return
    mod = types.ModuleType("antenv.axon_hooks")
    state = {"hook": None, "resolved": False}

    def set_axon_ntff_profile_hook(hook):
        state["hook"], state["resolved"] = hook, True

    def get_axon_ntff_profile_hook():
        if not state["resolved"]:
            state["resolved"] = True
            try:
                if "/root/.axon_site" not in sys.path:
                    sys.path.insert(0, "/root/.axon_site")
                from trn_agent_boot.trn_boot import _ntff_profile_via_ctypes
                state["hook"] = _ntff_profile_via_ctypes("/opt/axon/libaxon_pjrt.so")
            except Exception:
                state["hook"] = None
        return state["hook"]

    mod.set_axon_ntff_profile_hook = set_axon_ntff_profile_hook
    mod.get_axon_ntff_profile_hook = get_axon_ntff_profile_hook
    sys.modules["antenv.axon_hooks"] = mod
    antenv.axon_hooks = mod


def build_nc():
    import concourse.bacc as bacc
    import concourse.mybir as mybir
    from concourse import tile

    f32 = mybir.dt.float32
    bf16 = mybir.dt.bfloat16
    RG = [list(range(NCORES))]

    nc = bacc.Bacc("TRN2", target_bir_lowering=False, debug=False,
                   num_devices=NCORES)
    a_d = nc.dram_tensor("a", (K, C), bf16, kind="ExternalInput")
    w_d = nc.dram_tensor("w", (K, CB), bf16, kind="ExternalInput")
    qt_d = nc.dram_tensor("qt", (C, BB), bf16, kind="ExternalInput")
    ls_d = nc.dram_tensor("ls", (1, 1), f32, kind="ExternalInput")
    out_d = nc.dram_tensor("out", (BB, C), f32, kind="ExternalOutput")

    with tile.TileContext(nc) as tc:
        with tc.tile_pool(name="sbuf", bufs=1) as pool, \
             tc.tile_pool(name="psum", bufs=1, space="PSUM") as psum, \
             tc.tile_pool(name="dram", bufs=1, space="DRAM") as dram:
            fro_in = dram.tile([1, 1], f32)
            fro_out = dram.tile([1, 1], f32, addr_space="Shared")
            gin = [dram.tile([CB, CW], bf16, name=f"gin{n}")
                   for n in range(NCH)]
            gout = [dram.tile([C, CW], bf16, addr_space="Shared",
                              name=f"gout{n}") for n in range(NCH)]

            ls_sb = pool.tile([1, 1], f32, tag="ls")
            nc.gpsimd.dma_start(ls_sb[:], ls_d.ap()[:, :])

            # W resident: 32 k-tiles [128, CB] side by side (bf16)
            wt = pool.tile([128, KT * CB], bf16, tag="wt")
            for k in range(KT):
                eng = nc.sync if k % 2 == 0 else nc.scalar
                eng.dma_start(wt[:, k * CB:(k + 1) * CB],
                              w_d.ap()[k * 128:(k + 1) * 128, :])

            # query^T resident (bf16), issued on the scalar queue
            qt_sb = []
            for t in range(CT):
                qts = pool.tile([128, BB], bf16, tag=f"qt{t}", name=f"qts{t}")
                nc.scalar.dma_start(qts[:],
                                    qt_d.ap()[t * 128:(t + 1) * 128, :])
                qt_sb.append(qts)

            # ---- fro2 = sum(W^2) on DVE, then 4-byte AllReduce ----
            with nc.named_scope("fro2"):
                dp = pool.tile([128, 4], f32, tag="dp")
                for j in range(4):
                    sl = slice(j * 2048, (j + 1) * 2048)
                    sq = pool.tile([128, 2048], f32, tag="sq", bufs=1)
                    nc.vector.tensor_mul(sq[:], wt[:, sl], wt[:, sl])
                    nc.vector.reduce_sum(dp[:, j:j + 1], sq[:],
                                         axis=mybir.AxisListType.X)
                dx = pool.tile([128, 1], f32, tag="dx")
                nc.vector.reduce_sum(dx[:], dp[:], axis=mybir.AxisListType.X)
                fro_p = pool.tile([1, 1], f32, tag="frop")
                nc.gpsimd.tensor_reduce(fro_p[:], dx[:],
                                        op=mybir.AluOpType.add,
                                        axis=mybir.AxisListType.C)
                nc.scalar.dma_start(fro_in[:, :], fro_p[:])

            nc.gpsimd.collective_compute(
                "AllReduce", mybir.AluOpType.add, replica_groups=RG,
                ins=[fro_in.opt()], outs=[fro_out.opt()])

            # ---- GEMM1: G[R_i, cols_n] = W^T A[:, cols_n], chunk-wise ----
            with nc.named_scope("gemm1"):
                for n in range(NCH):
                    psg = [psum.tile([128, CW], f32, tag=f"ps{(n % 2) * 2 + m}",
                                     name=f"psg{n}_{m}") for m in range(2)]
                    for k in range(KT):
                        ak = pool.tile([128, CW], bf16, tag="ak", bufs=6)
                        eng = nc.sync if k % 2 == 0 else nc.scalar
                        eng.dma_start(ak[:],
                                      a_d.ap()[k * 128:(k + 1) * 128,
                                               n * CW:(n + 1) * CW])
                        for m in range(2):
                            nc.tensor.matmul(
                                psg[m][:],
                                wt[:, k * CB + m * 128:k * CB + (m + 1) * 128],
                                ak[:], start=(k == 0), stop=(k == KT - 1))
                    for m in range(2):
                        gsb = pool.tile([128, CW], bf16, tag="gsb", bufs=4)
                        nc.vector.tensor_copy(gsb[:], psg[m][:])
                        nc.sync.dma_start(gin[n][m * 128:(m + 1) * 128, :],
                                          gsb[:])
                    nc.gpsimd.collective_compute(
                        "AllGather", mybir.AluOpType.bypass, replica_groups=RG,
                        ins=[gin[n].opt()], outs=[gout[n].opt()])

            # ---- alpha chain: s = C1*min(exp(ls),clamp)/(fro2+1e-8) ----
            with nc.named_scope("alpha"):
                fro2 = pool.tile([1, 1], f32, tag="fro2")
                nc.scalar.dma_start(fro2[:], fro_out[:, :])
                ex = pool.tile([1, 1], f32, tag="ex")
                nc.scalar.activation(ex[:], ls_sb[:],
                                     mybir.ActivationFunctionType.Exp)
                emin = pool.tile([1, 1], f32, tag="emin")
                nc.vector.tensor_scalar_min(emin[:], ex[:], ALPHA_CLAMP)
                den = pool.tile([1, 1], f32, tag="den")
                nc.vector.tensor_scalar_add(den[:], fro2[:], 1e-8)
                r0 = pool.tile([1, 1], f32, tag="r0")
                nc.vector.reciprocal(r0[:], den[:])
                # one Newton step: r = r0*(2 - den*r0)
                t1 = pool.tile([1, 1], f32, tag="t1")
                nc.vector.tensor_mul(t1[:], den[:], r0[:])
                t2 = pool.tile([1, 1], f32, tag="t2")
                nc.vector.tensor_scalar(t2[:], t1[:], -1.0, 2.0,
                                        op0=mybir.AluOpType.mult,
                                        op1=mybir.AluOpType.add)
                rr = pool.tile([1, 1], f32, tag="rr")
                nc.vector.tensor_mul(rr[:], r0[:], t2[:])
                al = pool.tile([1, 1], f32, tag="al")
                nc.vector.tensor_mul(al[:], emin[:], rr[:])
                c1s = pool.tile([1, 1], f32, tag="c1s")
                nc.vector.tensor_scalar_mul(c1s[:], al[:], C1)
                c1b = pool.tile([128, 1], f32, tag="c1b")
                nc.gpsimd.partition_broadcast(c1b[:], c1s[:])

            # ---- GEMM3: out_i[:, cols_n] = Q_i @ G[:, cols_n], scaled ----
            with nc.named_scope("gemm3"):
                for n in range(NCH):
                    grh = []
                    for t in range(CT):
                        gr = pool.tile([128, CW], bf16, tag="gr", bufs=20)
                        eng = nc.sync if t % 2 == 0 else nc.scalar
                        eng.dma_start(gr[:],
                                      gout[n][t * 128:(t + 1) * 128, :])
                        grh.append(gr)
                    for m in range(BB // 128):
                        po = psum.tile([128, CW], f32, tag=f"ps{4 + m % 4}",
                                       name=f"po{n}_{m}")
                        for t in range(CT):
                            nc.tensor.matmul(
                                po[:], qt_sb[t][:, m * 128:(m + 1) * 128],
                                grh[t][:], start=(t == 0), stop=(t == CT - 1))
                        osb = pool.tile([128, CW], f32, tag="osb", bufs=4)
                        nc.vector.tensor_scalar_mul(osb[:], po[:], c1b[:])
                        eng = nc.scalar if m % 2 == 0 else nc.sync
                        eng.dma_start(out_d.ap()[m * 128:(m + 1) * 128,
                                                 n * CW:(n + 1) * CW], osb[:])
    nc.compile()
    return nc


def _get_nc():
    if "nc" not in _CACHE:
        _CACHE["nc"] = build_nc()
    return _CACHE["nc"]


def _run(query, memory_mean, ben_israel_log_scale, trace=False, trace_cores=None):
    import ml_dtypes
    from concourse import bass_utils

    _install_ntff_shim()
    nc = _get_nc()

    bf = ml_dtypes.bfloat16
    a_bf = np.ascontiguousarray(np.asarray(memory_mean, dtype=np.float32)
                                .astype(bf))
    q_bf = np.asarray(query, dtype=np.float32).astype(bf)
    ls = np.asarray(ben_israel_log_scale, dtype=np.float32).reshape(1, 1)

    in_maps = []
    for i in range(NCORES):
        in_maps.append({
            "a": a_bf,
            "w": np.ascontiguousarray(a_bf[:, i * CB:(i + 1) * CB]),
            "qt": np.ascontiguousarray(q_bf[i * BB:(i + 1) * BB, :].T),
            "ls": ls,
        })
    res = bass_utils.run_bass_kernel_spmd(
        nc, in_maps, core_ids=list(range(NCORES)), trace=trace,
        trace_cores=trace_cores)
    out = np.concatenate([res.results[i]["out"] for i in range(NCORES)], axis=0)
    return out, res


def kernel(query, memory_mean, ben_israel_log_scale):
    out, _ = _run(query, memory_mean, ben_israel_log_scale, trace=False)
    return out


# revision 8
# speedup vs baseline: 2.1979x; 1.1088x over previous
"""Trainium2 Bass kernel for nn_CA3RecurrentMatrix (scatter_memory).

Math: the reference's Ben-Israel-Cohen pseudoinverse iteration collapses
algebraically.  With pinv_0 = alpha*A^T, every iterate is pinv_n = p_n(G) A^T
with G = A^T A (C x C), and on eigenvalues g of G the output polynomial is
u_8(g)*g = 1 - (1 - alpha*g)^256 = 256*alpha*g - C(256,2)*(alpha*g)^2 + ...
Because alpha <= 5e-4/||A||_F^2, alpha*g_max ~ 7e-7, so even the QUADRATIC
term is <1e-4 relative (verified numerically: dropping it gives 5.7e-5 max
rel err).  Hence, to well within the 2e-2 gate:

    out = (256*alpha) * query @ (A^T A)

Distribution over 8 cores, all bf16 compute (validated 4.2e-3 max rel err):
core i holds W_i = A[:, R_i] (bf16) and computes G row-block
G[R_i, :] = W_i^T A in four 512-column chunks; each chunk is AllGathered
(bf16, ~2MB) as soon as it is ready, and the big GEMM out_i = Q_i @ G
consumes gathered column-chunks as they land.  The partial Frobenius sum
sum(W_i^2) rides as one extra payload row in the FIRST AllGather (f32
bit-split into two bf16 slots), so there is no separate AllReduce on the
serial CC ring; the scale s = 256*min(exp(ls),5e-4)/(fro2+1e-8) is applied
during the PSUM->SBUF eviction of the output tiles.  Queue discipline:
sync = W + A-stream + out stores, scalar = QT + gin stores, gpsimd = AG
triggers + gathered-G loads (HOL-blocking there is free since the CC ring
serializes AllGathers anyway).
"""
import sys, os, types

sys.path.insert(0, "/opt/trn_rl_repo")

import numpy as np

B, C, K = 8192, 2048, 4096
NCORES = 8
CB = C // NCORES     # 256 G-row block per core
BB = B // NCORES     # 1024 query rows per core
KT = K // 128        # 32 k-tiles over K
CT = C // 128        # 16 c-tiles over C
CW = 512             # column-chunk width
NCH = C // CW        # 4 column chunks
ALPHA_CLAMP = 5e-4
C1 = 256.0

_CACHE = {}


def _install_ntff_shim():
    """Make trace=True work under axon (antenv.axon_hooks is absent here)."""
    if "antenv.axon_hooks" in sys.modules:
        return
    try:
        import antenv
    except ImportError:
        return
    mod = types.ModuleType("antenv.axon_hooks")
    state = {"hook": None, "resolved": False}

    def set_axon_ntff_profile_hook(hook):
        state["hook"], state["resolved"] = hook, True

    def get_axon_ntff_profile_hook():
        if not state["resolved"]:
            state["resolved"] = True
            try:
                if "/root/.axon_site" not in sys.path:
                    sys.path.insert(0, "/root/.axon_site")
                from trn_agent_boot.trn_boot import _ntff_profile_via_ctypes
                state["hook"] = _ntff_profile_via_ctypes("/opt/axon/libaxon_pjrt.so")
            except Exception:
                state["hook"] = None
        return state["hook"]

    mod.set_axon_ntff_profile_hook = set_axon_ntff_profile_hook
    mod.get_axon_ntff_profile_hook = get_axon_ntff_profile_hook
    sys.modules["antenv.axon_hooks"] = mod
    antenv.axon_hooks = mod


def build_nc():
    import concourse.bacc as bacc
    import concourse.mybir as mybir
    from concourse import tile

    f32 = mybir.dt.float32
    bf16 = mybir.dt.bfloat16
    RG = [list(range(NCORES))]

    nc = bacc.Bacc("TRN2", target_bir_lowering=False, debug=False,
                   num_devices=NCORES)
    a_d = nc.dram_tensor("a", (K, C), bf16, kind="ExternalInput")
    # host pre-swizzled: [128, KT*CB] so rows are 16KB contiguous
    w_d = nc.dram_tensor("w", (128, KT * CB), bf16, kind="ExternalInput")
    qt_d = nc.dram_tensor("qt", (C, BB), bf16, kind="ExternalInput")
    ls_d = nc.dram_tensor("ls", (128, 1), f32, kind="ExternalInput")
    out_d = nc.dram_tensor("out", (BB, C), f32, kind="ExternalOutput")

    with tile.TileContext(nc) as tc:
        with tc.tile_pool(name="sbuf", bufs=1) as pool, \
             tc.tile_pool(name="psum", bufs=1, space="PSUM") as psum, \
             tc.tile_pool(name="dram", bufs=1, space="DRAM") as dram:
            # chunk 0 carries one extra payload row per rank: the partial
            # Frobenius sum (f32 bit-split into two bf16 slots)
            gin0 = dram.tile([CB + 1, CW], bf16, name="gin0")
            gout0 = dram.tile([(CB + 1) * NCORES, CW], bf16,
                              addr_space="Shared", name="gout0")
            gin = [gin0] + [dram.tile([CB, CW], bf16, name=f"gin{n}")
                            for n in range(1, NCH)]
            gout = [gout0] + [dram.tile([C, CW], bf16, addr_space="Shared",
                                        name=f"gout{n}")
                              for n in range(1, NCH)]

            ls_sb = pool.tile([128, 1], f32, tag="ls")
            nc.gpsimd.dma_start(ls_sb[:], ls_d.ap()[:, :])
            # exp(ls) early on the scalar queue (before QT) so the alpha
            # chain never waits on scalar-queue HOL
            ex = pool.tile([128, 1], f32, tag="ex")
            nc.scalar.activation(ex[:], ls_sb[:],
                                 mybir.ActivationFunctionType.Exp)

            # W resident: [128, KT*CB], k-tile k at cols [k*CB:(k+1)*CB].
            # 4 DMAs so GEMM1 k-tile 0 isn't gated on the whole 2MB.
            wt = pool.tile([128, KT * CB], bf16, tag="wt")
            for j in range(4):
                sl = slice(j * 8 * CB, (j + 1) * 8 * CB)
                nc.sync.dma_start(wt[:, sl], w_d.ap()[:, sl])

            # PE warm-up: garbage matmuls on the first W quarter to lift the
            # HAM clock gate while the A-stream lands (never read back)
            psw = psum.tile([128, 512], f32, tag="ps7", name="psw")
            for _ in range(8):
                nc.tensor.matmul(psw[:], wt[:, 0:128], wt[:, 0:512],
                                 start=True, stop=True)

            # query^T resident (bf16) — first on the scalar queue
            qt_sb = []
            for t in range(CT):
                qts = pool.tile([128, BB], bf16, tag=f"qt{t}", name=f"qts{t}")
                nc.scalar.dma_start(qts[:],
                                    qt_d.ap()[t * 128:(t + 1) * 128, :])
                qt_sb.append(qts)

            # ---- partial fro2 = sum(W_i^2) on DVE ----
            with nc.named_scope("fro2"):
                dp = pool.tile([128, 4], f32, tag="dp")
                for j in range(4):
                    sl = slice(j * 8 * CB, (j + 1) * 8 * CB)
                    sq = pool.tile([128, 8 * CB], f32, tag="sq", bufs=1)
                    nc.vector.tensor_mul(sq[:], wt[:, sl], wt[:, sl])
                    nc.vector.reduce_sum(dp[:, j:j + 1], sq[:],
                                         axis=mybir.AxisListType.X)
                dx = pool.tile([128, 1], f32, tag="dx")
                nc.vector.reduce_sum(dx[:], dp[:], axis=mybir.AxisListType.X)
                fro_p = pool.tile([1, 1], f32, tag="frop")
                nc.gpsimd.tensor_reduce(fro_p[:], dx[:],
                                        op=mybir.AluOpType.add,
                                        axis=mybir.AxisListType.C)
                # ride along in AllGather chunk 0 (bit-exact f32 in 2 bf16)
                nc.scalar.dma_start(gin0[CB:CB + 1, 0:2].bitcast(f32),
                                    fro_p[:])

            # ---- GEMM1: G[R_i, cols_n] = W^T A[:, cols_n], chunk-wise;
            #      AllGather each chunk as soon as it is stored ----
            with nc.named_scope("gemm1"):
                grhs_all = []
                for n in range(NCH):
                    psg = [psum.tile([128, CW], f32, tag=f"ps{(n % 2) * 2 + m}",
                                     name=f"psg{n}_{m}") for m in range(2)]
                    for k in range(KT):
                        ak = pool.tile([128, CW], bf16, tag="ak", bufs=8)
                        nc.sync.dma_start(ak[:],
                                          a_d.ap()[k * 128:(k + 1) * 128,
                                                   n * CW:(n + 1) * CW])
                        for m in range(2):
                            nc.tensor.matmul(
                                psg[m][:],
                                wt[:, k * CB + m * 128:k * CB + (m + 1) * 128],
                                ak[:], start=(k == 0), stop=(k == KT - 1))
                    for m in range(2):
                        gsb = pool.tile([128, CW], bf16, tag="gsb", bufs=4)
                        nc.vector.tensor_copy(gsb[:], psg[m][:])
                        nc.scalar.dma_start(gin[n][m * 128:(m + 1) * 128, :],
                                            gsb[:])
                    nc.gpsimd.collective_compute(
                        "AllGather", mybir.AluOpType.bypass, replica_groups=RG,
                        ins=[gin[n].opt()], outs=[gout[n].opt()])
                    # gathered-G loads ride the gpsimd queue: they only wait
                    # on AG_n, and blocking AG_{n+1}'s trigger is free since
                    # the CC ring serializes AllGathers anyway
                    grh = []
                    for t in range(CT):
                        gr = pool.tile([128, CW], bf16, tag=f"gr{n}", bufs=CT,
                                       name=f"gr{n}_{t}")
                        if n == 0:
                            row0 = (t // 2) * (CB + 1) + (t % 2) * 128
                        else:
                            row0 = t * 128
                        nc.gpsimd.dma_start(gr[:],
                                            gout[n][row0:row0 + 128, :])
                        grh.append(gr)
                    grhs_all.append(grh)
                    if n == 0:
                        # extract the 8 partial-fro rows from gout0
                        tr_sb = pool.tile([NCORES, 2], bf16, tag="trsb")
                        for r in range(NCORES):
                            row = r * (CB + 1) + CB
                            nc.gpsimd.dma_start(
                                tr_sb[r:r + 1, :], gout0[row:row + 1, 0:2])
                        fro2 = pool.tile([1, 1], f32, tag="fro2")
                        nc.gpsimd.tensor_reduce(
                            fro2[:], tr_sb[:, 0:2].bitcast(f32),
                            op=mybir.AluOpType.add, axis=mybir.AxisListType.C)
                        fro2b = pool.tile([128, 1], f32, tag="fro2b")
                        nc.gpsimd.partition_broadcast(fro2b[:], fro2[:])

            # ---- alpha chain: s = C1*min(exp(ls),clamp)/(fro2+1e-8),
            #      computed as [128,1] elementwise so no late broadcast ----
            with nc.named_scope("alpha"):
                emin = pool.tile([128, 1], f32, tag="emin")
                nc.vector.tensor_scalar_min(emin[:], ex[:], ALPHA_CLAMP)
                den = pool.tile([128, 1], f32, tag="den")
                nc.vector.tensor_scalar_add(den[:], fro2b[:], 1e-8)
                r0 = pool.tile([128, 1], f32, tag="r0")
                nc.vector.reciprocal(r0[:], den[:])
                # one Newton step: r = r0*(2 - den*r0)
                t1 = pool.tile([128, 1], f32, tag="t1")
                nc.vector.tensor_mul(t1[:], den[:], r0[:])
                t2 = pool.tile([128, 1], f32, tag="t2")
                nc.vector.tensor_scalar(t2[:], t1[:], -1.0, 2.0,
                                        op0=mybir.AluOpType.mult,
                                        op1=mybir.AluOpType.add)
                rr = pool.tile([128, 1], f32, tag="rr")
                nc.vector.tensor_mul(rr[:], r0[:], t2[:])
                al = pool.tile([128, 1], f32, tag="al")
                nc.vector.tensor_mul(al[:], emin[:], rr[:])
                c1b = pool.tile([128, 1], f32, tag="c1b")
                nc.vector.tensor_scalar_mul(c1b[:], al[:], C1)

            # ---- GEMM3: out_i[:, cols_n] = Q_i @ G[:, cols_n], scaled ----
            with nc.named_scope("gemm3"):
                for n in range(NCH):
                    grh = grhs_all[n]
                    for m in range(BB // 128):
                        po = psum.tile([128, CW], f32, tag=f"ps{4 + m % 4}",
                                       name=f"po{n}_{m}")
                        for t in range(CT):
                            nc.tensor.matmul(
                                po[:], qt_sb[t][:, m * 128:(m + 1) * 128],
                                grh[t][:], start=(t == 0), stop=(t == CT - 1))
                        osb = pool.tile([128, CW], f32, tag="osb", bufs=4)
                        nc.vector.tensor_scalar_mul(osb[:], po[:], c1b[:])
                        nc.sync.dma_start(out_d.ap()[m * 128:(m + 1) * 128,
                                                     n * CW:(n + 1) * CW],
                                          osb[:])
    nc.compile()
    return nc


def _get_nc():
    if "nc" not in _CACHE:
        _CACHE["nc"] = build_nc()
    return _CACHE["nc"]


def _run(query, memory_mean, ben_israel_log_scale, trace=False, trace_cores=None):
    import ml_dtypes
    from concourse import bass_utils

    _install_ntff_shim()
    nc = _get_nc()

    bf = ml_dtypes.bfloat16
    a_bf = np.ascontiguousarray(np.asarray(memory_mean, dtype=np.float32)
                                .astype(bf))
    q_bf = np.asarray(query, dtype=np.float32).astype(bf)
    ls = np.full((128, 1), np.float32(np.asarray(ben_israel_log_scale)),
                 dtype=np.float32)

    in_maps = []
    for i in range(NCORES):
        w_sw = np.ascontiguousarray(
            a_bf[:, i * CB:(i + 1) * CB].reshape(KT, 128, CB)
            .transpose(1, 0, 2).reshape(128, KT * CB))
        in_maps.append({
            "a": a_bf,
            "w": w_sw,
            "qt": np.ascontiguousarray(q_bf[i * BB:(i + 1) * BB, :].T),
            "ls": ls,
        })
    res = bass_utils.run_bass_kernel_spmd(
        nc, in_maps, core_ids=list(range(NCORES)), trace=trace,
        trace_cores=trace_cores)
    out = np.concatenate([res.results[i]["out"] for i in range(NCORES)], axis=0)
    return out, res


def kernel(query, memory_mean, ben_israel_log_scale):
    out, _ = _run(query, memory_mean, ben_israel_log_scale, trace=False)
    return out
